# revision 1
# baseline (speedup 1.0000x reference)
"""Fused causal-attention block (QKV proj + causal softmax attention + out proj
+ residual + LayerNorm) on 8 Trainium2 NeuronCores — bf16 v2.

Sharding: core c -> batch b = c//4, head-group r = c%4 (heads 4r..4r+3, local
model dims 256r..256r+256).  Each core computes Q/K/V for its head group over
its batch's full sequence and block-causal attention (no max subtraction --
scores are O(1)).  Output projection is row-parallel: each core computes the
partial out-proj Y_r = ctx_r @ Wo[:, 256r:256r+256].T for ALL 2048 rows from
its local (normalized) ctx, per 512-row q-tile, overlapped with attention;
a per-q-tile ReduceScatter over the batch's 4 cores sums the partials and
hands each core a 128-row shard, on which it does residual + LayerNorm.
Host reassembles the 8 x [4, 128, 1024] shards.

All matmul operands are bf16 (rel err ~1e-3, gate is 2e-2); PSUM accumulation
is fp32.  The causal mask on diagonal 128x128 blocks is a bf16 upper-tri
multiply applied to exp(scores) on the GPSIMD engine (SBUF-only op, keeps DVE
free).  Softmax denominators come from an all-ones column appended to V; the
[1,512]-per-head reciprocals use the fast custom-DVE approx (~5x faster than
InstReciprocal).  LayerNorm rstd = exp(-0.5*ln(var+eps)) so the Scalar engine
stays on the natural_log_exp activation-table set for the whole kernel (no
table thrash against the attention exp stream).
"""

import numpy as np

B, N, D = 2, 2048, 1024
H, DH = 16, 64
NCORES = 8
HPC = 4          # heads per core
DP = HPC * DH    # 256 local model dims per core
NQ = N // 4      # 512 rows per q-tile
LN_EPS = 1e-5
GROUPS = [[0, 1, 2, 3], [4, 5, 6, 7]]

_CACHE = {}


def _build(flags):
    """Build+compile the Bacc program. flags = (has_qkv_bias, has_gamma, has_beta)."""
    import concourse.bass as bass
    import concourse.bacc as bacc
    import concourse.tile as tile
    from concourse import mybir
    from contextlib import ExitStack

    has_qkv_bias, has_gamma, has_beta = flags
    f32 = mybir.dt.float32
    f32r = mybir.dt.float32r
    bf16 = mybir.dt.bfloat16
    AF = mybir.ActivationFunctionType
    ALU = mybir.AluOpType

    nc = bacc.Bacc(
        trn_type="TRN2",
        target_bir_lowering=False,
        debug=False,
        num_devices=NCORES,
    )

    xT = nc.dram_tensor("xT", [D, N], bf16, kind="ExternalInput").ap()
    xres = nc.dram_tensor("xres", [4, 128, D], bf16, kind="ExternalInput").ap()
    wqT = nc.dram_tensor("wqT", [D, DP], bf16, kind="ExternalInput").ap()
    wkT = nc.dram_tensor("wkT", [D, DP], bf16, kind="ExternalInput").ap()
    wvT = nc.dram_tensor("wvT", [D, DP], bf16, kind="ExternalInput").ap()
    woL = nc.dram_tensor("woL", [DP, D], bf16, kind="ExternalInput").ap()
    out = nc.dram_tensor("out", [4, 128, D], f32, kind="ExternalOutput").ap()
    if has_qkv_bias:
        bqkv = nc.dram_tensor("bqkv", [1, 3, DP], bf16, kind="ExternalInput").ap()
    if has_gamma:
        gamma_d = nc.dram_tensor("gamma", [D], f32, kind="ExternalInput").ap()
    if has_beta:
        beta_d = nc.dram_tensor("beta", [D], f32, kind="ExternalInput").ap()

    # multiplicative causal mask for diagonal blocks: keep k <= q
    # (partition p = k offset, free c = q offset)
    import ml_dtypes
    tri_np = np.triu(np.ones((128, 128), np.float32)).astype(ml_dtypes.bfloat16)
    tri_d = nc.inline_tensor(tri_np, name="tri_const").ap()

    with tile.TileContext(nc) as tc, ExitStack() as ctx, \
            nc.allow_low_precision(reason="bf16 compute; gate is 2e-2"):
        singles = ctx.enter_context(tc.tile_pool(name="singles", bufs=1))
        qkv_pool = ctx.enter_context(tc.tile_pool(name="qkv", bufs=1))

        # weights, striped k-on-partitions
        wq_sb = singles.tile([128, 8, DP], bf16, tag="wq")
        wk_sb = singles.tile([128, 8, DP], bf16, tag="wk")
        wv_sb = singles.tile([128, 8, DP], bf16, tag="wv")
        nc.scalar.dma_start(wq_sb, wqT.rearrange("(ko p) m -> p ko m", p=128))
        nc.gpsimd.dma_start(wk_sb, wkT.rearrange("(ko p) m -> p ko m", p=128))
        nc.gpsimd.dma_start(wv_sb, wvT.rearrange("(ko p) m -> p ko m", p=128))

        tri_sb = singles.tile([128, 128], bf16, tag="tri")

        ones_f32 = singles.tile([128, 64], f32, tag="ones_f32")
        nc.vector.memset(ones_f32, 1.0)
        ones64 = singles.tile([1, 64], bf16, tag="ones64")
        nc.vector.tensor_copy(out=ones64, in_=ones_f32[0:1, :])
        eps_sb = singles.tile([128, 1], f32, tag="eps")
        nc.vector.memset(eps_sb, LN_EPS)
        if has_qkv_bias:
            o512f = singles.tile([1, 512], f32, tag="o512f")
            nc.vector.memset(o512f, 1.0)
            ones512 = singles.tile([1, 512], bf16, tag="ones512")
            nc.vector.tensor_copy(out=ones512, in_=o512f)
            bqkv_sb = singles.tile([1, 3, DP], bf16, tag="bqkv")
            nc.scalar.dma_start(bqkv_sb, bqkv)
        if has_gamma:
            gamma_sb = singles.tile([128, D], f32, tag="gamma")
            nc.scalar.dma_start(
                gamma_sb,
                bass.AP(tensor=gamma_d.tensor, offset=gamma_d.offset,
                        ap=[[0, 128]] + gamma_d.ap),
            )
        if has_beta:
            beta_sb = singles.tile([128, D], f32, tag="beta")
            nc.scalar.dma_start(
                beta_sb,
                bass.AP(tensor=beta_d.tensor, offset=beta_d.offset,
                        ap=[[0, 128]] + beta_d.ap),
            )

        # persistent activations
        qT_sb = qkv_pool.tile([128, 2, N], bf16, tag="qT")   # Q^T [d'(256), n]
        kT_sb = qkv_pool.tile([128, 2, N], bf16, tag="kT")   # K^T [d'(256), n]
        v_sb = qkv_pool.tile([128, 16, HPC, DH + 2], bf16, tag="v")  # V + ones
        ctx_sb = qkv_pool.tile([128, 2, N], bf16, tag="ctxT")  # normalized ctx^T
        # The AV stationary reads the FULL 66-wide inner dim (odd-width
        # partial slices break HW ldweights addressing): zero the tile, set
        # the denominator ones column; col 65 stays 0 so its output row is
        # junk-free and ignored.
        nc.vector.memset(v_sb, 0.0)
        nc.vector.tensor_copy(
            out=v_sb[:, :, :, DH:DH + 1],
            in_=ones_f32.rearrange("p (a b c) -> p a b c", a=16, b=4))

        # ---------------- Phase 1: QKV projections ----------------
        with tc.tile_pool(name="xt", bufs=1) as xt_pool, \
             tc.tile_pool(name="p1qk", bufs=4, space="PSUM") as p1qk, \
             tc.tile_pool(name="p1v", bufs=3, space="PSUM") as p1v:
            xT_sb = xt_pool.tile([128, 8, N], bf16, tag="xT")
            xT_r = xT.rearrange("(ko p) n -> p ko n", p=128)
            # 4 column chunks in nt-consumption order, few enough early DMAs
            # (<=8) that each gets its own completion-semaphore slot and the
            # first Q matmuls aren't gated on later chunks
            dma_engs = [nc.sync, nc.scalar, nc.gpsimd, nc.sync]
            for ci in range(4):
                dma_engs[ci].dma_start(
                    xT_sb[:, :, 512 * ci:512 * (ci + 1)],
                    xT_r[:, :, 512 * ci:512 * (ci + 1)])
            nc.scalar.dma_start(tri_sb, tri_d)

            for wsb, dst, bidx in ((wq_sb, qT_sb, 0), (wk_sb, kT_sb, 1)):
                for dt_ in range(2):
                    for nt in range(4):
                        ps = p1qk.tile([128, 512], f32, tag="qk")
                        for ko in range(8):
                            nc.tensor.matmul(
                                ps,
                                lhsT=wsb[:, ko, 128 * dt_:128 * dt_ + 128],
                                rhs=xT_sb[:, ko, 512 * nt:512 * nt + 512],
                                start=(ko == 0),
                                stop=(ko == 7 and not has_qkv_bias),
                            )
                        if has_qkv_bias:
                            nc.tensor.matmul(
                                ps,
                                lhsT=bqkv_sb[:, bidx, 128 * dt_:128 * dt_ + 128],
                                rhs=ones512,
                                start=False, stop=True,
                            )
                        nc.vector.tensor_copy(
                            out=dst[:, dt_, 512 * nt:512 * (nt + 1)], in_=ps)

            for nt in range(16):
                ps = p1v.tile([128, DP], f32, tag="v")
                for ko in range(8):
                    nc.tensor.matmul(
                        ps,
                        lhsT=xT_sb[:, ko, 128 * nt:128 * nt + 128],
                        rhs=wv_sb[:, ko],
                        start=(ko == 0),
                        stop=(ko == 7 and not has_qkv_bias),
                    )
                if has_qkv_bias:
                    nc.tensor.matmul(
                        ps,
                        lhsT=ones512[:, 0:128],
                        rhs=bqkv_sb[:, 2, :],
                        start=False, stop=True,
                    )
                nc.vector.tensor_copy(
                    out=v_sb[:, nt, :, 0:DH],
                    in_=ps.rearrange("p (h d) -> p h d", h=HPC))

        # local Wo^T slice + residual rows + LN params: small loads that
        # overlap attention
        wo_pool = ctx.enter_context(tc.tile_pool(name="wop", bufs=1))
        wo_sb = wo_pool.tile([128, 2, D], bf16, tag="wo")
        nc.scalar.dma_start(wo_sb, woL.rearrange("(c p) d -> p c d", p=128))
        xres_sb = wo_pool.tile([128, 4, D], bf16, tag="xres")
        nc.gpsimd.dma_start(
            xres_sb, xres.rearrange("a p d -> p a d"))

        dram_pool = ctx.enter_context(tc.tile_pool(name="dram", bufs=1,
                                                   space="DRAM"))
        y_dram = [dram_pool.tile([NQ, D], bf16, tag=f"y{qt}", name=f"y{qt}")
                  for qt in range(4)]
        yr_dram = [dram_pool.tile([128, D], bf16, tag=f"yr{qt}",
                                  name=f"yr{qt}")
                   for qt in range(4)]

        # ------------- Phase 2: attention + overlapped out-proj -------------
        with tc.tile_pool(name="es", bufs=6) as es_pool, \
             tc.tile_pool(name="nrm", bufs=4) as nrm_pool, \
             tc.tile_pool(name="ysb", bufs=2) as y_pool, \
             tc.tile_pool(name="ln", bufs=2) as ln_pool, \
             tc.tile_pool(name="sps", bufs=2, space="PSUM") as sps_pool, \
             tc.tile_pool(name="cps", bufs=2, space="PSUM") as cps_pool:

            from concourse.dve_ops import (
                RECIP_APPROX_FAST_CONSTS,
                RECIPROCAL_APPROX_FAST,
            )

            def emit_recip(qt, hp, cps):
                # Evacuate the denominator row to partition 0 of SBUF (the
                # custom-DVE op ignores input base partitions on HW), then
                # 1/denoms for both heads in one fast custom-DVE op (~5x
                # faster than InstReciprocal), cast to bf16 for the
                # broadcast matmul.
                den = nrm_pool.tile([1, 2, 512], f32, tag="den",
                                    name=f"den_{qt}_{hp}")
                nc.vector.tensor_copy(out=den, in_=cps[64:65, :, :])
                recf = nrm_pool.tile([1, 2, 512], f32, tag="recf",
                                     name=f"recf_{qt}_{hp}")
                cc = RECIP_APPROX_FAST_CONSTS
                nc.vector._custom_dve(
                    RECIPROCAL_APPROX_FAST, out=recf, in0=den,
                    s0=cc["s0"], s1=cc["s1"], imm2=cc["imm2"])
                rec = nrm_pool.tile([1, 2, 512], bf16, tag="rec",
                                    name=f"rec_{qt}_{hp}")
                nc.vector.tensor_copy(out=rec, in_=recf)
                return rec

            def emit_normalize(qt, hp, cps, rec):
                bc_full = sps_pool.tile([128, 2, 512], f32, tag="s",
                                        name=f"bc_{qt}_{hp}")
                bcs = nrm_pool.tile([64, 2, 512], bf16, tag="bcs",
                                    name=f"bcs_{qt}_{hp}")
                for hi in range(2):
                    nc.tensor.matmul(
                        bc_full[0:64, hi, :], lhsT=ones64,
                        rhs=rec[0:1, hi, :],
                        start=True, stop=True)
                    nc.vector.tensor_copy(out=bcs[:, hi, :],
                                          in_=bc_full[0:64, hi, :])
                for hi, h in enumerate((2 * hp, 2 * hp + 1)):
                    ph = 64 * (h % 2)
                    nc.vector.tensor_mul(
                        out=ctx_sb[ph:ph + 64, hp, 512 * qt:512 * (qt + 1)],
                        in0=cps[0:64, hi, :], in1=bcs[:, hi, :])

            def emit_outproj(qt):
                # partial out-proj for this q-tile from local normalized ctx;
                # stage to DRAM and ReduceScatter across the batch group
                y_sb = y_pool.tile([128, 4, D], bf16, tag="y",
                                   name=f"y_sb{qt}")
                for qc in range(4):
                    ops = sps_pool.tile([128, 2, 512], f32, tag="s",
                                        name=f"op_{qt}_{qc}")
                    for Dt in range(2):
                        for hp in range(2):
                            nc.tensor.matmul(
                                ops[:, Dt, :],
                                lhsT=ctx_sb[:, hp,
                                            512 * qt + 128 * qc:
                                            512 * qt + 128 * qc + 128],
                                rhs=wo_sb[:, hp, 512 * Dt:512 * Dt + 512],
                                start=(hp == 0), stop=(hp == 1),
                            )
                    nc.vector.tensor_copy(
                        out=y_sb[:, qc, :],
                        in_=ops.rearrange("p a b -> p (a b)"))
                nc.sync.dma_start(
                    y_dram[qt].rearrange("(qc p) d -> p qc d", p=128), y_sb)
                nc.gpsimd.collective_compute(
                    "ReduceScatter", ALU.add,
                    replica_groups=GROUPS,
                    ins=[y_dram[qt][:, :]],
                    outs=[yr_dram[qt][:, :]],
                )

            def emit_lntail(qt):
                yr_sb = ln_pool.tile([128, D], bf16, tag="yr",
                                     name=f"yr_sb{qt}")
                nc.sync.dma_start(yr_sb, yr_dram[qt])
                yt = ln_pool.tile([128, D], f32, tag="yt", name=f"yt{qt}")
                nc.vector.tensor_add(out=yt, in0=yr_sb, in1=xres_sb[:, qt])
                st = ln_pool.tile([128, 2, 6], f32, tag="st", name=f"st{qt}")
                nc.vector.bn_stats(out=st[:, 0], in_=yt[:, 0:512])
                nc.vector.bn_stats(out=st[:, 1], in_=yt[:, 512:1024])
                mv = ln_pool.tile([128, 2], f32, tag="mv", name=f"mv{qt}")
                nc.vector.bn_aggr(out=mv, in_=st)
                # rstd = exp(-0.5*ln(var+eps)): stays on the natural_log_exp
                # activation table set (same set as the attention exp)
                lnt = ln_pool.tile([128, 1], f32, tag="lnt", name=f"lnt{qt}")
                rstd = ln_pool.tile([128, 1], f32, tag="rstd",
                                    name=f"rstd{qt}")
                nc.scalar.activation(out=lnt, in_=mv[:, 1:2], func=AF.Ln,
                                     bias=eps_sb, scale=1.0)
                nc.scalar.activation(out=rstd, in_=lnt, func=AF.Exp,
                                     scale=-0.5)
                nc.vector.tensor_scalar(
                    out=yt, in0=yt, scalar1=mv[:, 0:1], scalar2=rstd,
                    op0=ALU.subtract, op1=ALU.mult)
                if has_gamma:
                    nc.vector.tensor_mul(out=yt, in0=yt, in1=gamma_sb)
                if has_beta:
                    nc.vector.tensor_add(out=yt, in0=yt, in1=beta_sb)
                nc.sync.dma_start(out[qt], yt)

            # deferred-work queue: (min_pair_idx, fn) emitted at staggered
            # points inside later pairs' kt loops so the in-order PE never
            # drains behind DVE/collective latencies
            deferred = []

            def pump(pidx):
                for i, (minp, fn) in enumerate(deferred):
                    if minp <= pidx:
                        deferred.pop(i)
                        fn()
                        return

            for qt in range(4):
                for hp in range(2):
                    pidx = 2 * qt + hp
                    heads = (2 * hp, 2 * hp + 1)
                    cps = cps_pool.tile([128, 2, 512], f32, tag="ctx",
                                        name=f"cps_{qt}_{hp}")
                    n_kt = 4 * qt + 4
                    milestones = sorted({2, n_kt // 2, n_kt - 2})
                    pend = []
                    for kt in range(n_kt):
                        j = kt - 4 * qt
                        c0 = 128 * j if j > 0 else 0
                        sp = sps_pool.tile([128, 2, 512], f32, tag="s",
                                           name=f"sp_{qt}_{hp}_{kt}")
                        for hi, h in enumerate(heads):
                            ph = 64 * (h % 2)
                            nc.tensor.matmul(
                                sp[:, hi, c0:512],
                                lhsT=kT_sb[ph:ph + 64, hp,
                                           128 * kt:128 * kt + 128],
                                rhs=qT_sb[ph:ph + 64, hp,
                                          512 * qt + c0:512 * (qt + 1)],
                                start=True,
                                stop=True,
                            )
                        es = es_pool.tile([128, 2, 512], bf16, tag="es")
                        nc.scalar.activation(
                            out=es[:, :, c0:512],
                            in_=sp[:, :, c0:512],
                            func=AF.Exp, scale=0.125,
                        )
                        if j >= 0:
                            for hi in range(2):
                                nc.gpsimd.tensor_mul(
                                    out=es[:, hi, 128 * j:128 * j + 128],
                                    in0=es[:, hi, 128 * j:128 * j + 128],
                                    in1=tri_sb)
                        pend.append((es, kt, c0))
                        if len(pend) > 2:
                            _emit_av(nc, cps, v_sb, heads, pend.pop(0), n_kt)
                        if kt in milestones:
                            pump(pidx)
                    while pend:
                        _emit_av(nc, cps, v_sb, heads, pend.pop(0), n_kt)
                    rec = emit_recip(qt, hp, cps)
                    deferred.append(
                        (pidx + 1,
                         (lambda a, b, c, d: lambda: emit_normalize(a, b, c, d))(
                             qt, hp, cps, rec)))
                    if hp == 1:
                        deferred.append(
                            (pidx + 1, (lambda a: lambda: emit_outproj(a))(qt)))
            while deferred:
                deferred.sort(key=lambda it: it[0])
                _, fn = deferred.pop(0)
                fn()
            # all residual+LN tails at the end: RS(0..2) wires are long done,
            # so no mid-stream engine ever blocks on a collective semaphore
            for qt in range(4):
                emit_lntail(qt)

    # Pin every activation to the natural_log_exp table set (covers exp, ln,
    # copy) so the Scalar engine never swaps tables between the attention exp
    # stream and the LayerNorm rstd (each swap costs ~1.5us + a pipe drain).
    import concourse.bacc as bacc_mod
    orig_tables = bacc_mod.get_activation_tables
    # keep dict insertion order (act_func_set_id is positional) but leave
    # functions only in the one set we want chosen
    bacc_mod.get_activation_tables = lambda arch: {
        k: (v if k == "natural_log_exp_and_others" else set())
        for k, v in orig_tables(arch).items()
    }
    try:
        nc.compile()
    finally:
        bacc_mod.get_activation_tables = orig_tables
    return nc


def _emit_av(nc, cps, v_sb, heads, pend_item, n_kt):
    es, kt, c0 = pend_item
    for hi, h in enumerate(heads):
        nc.tensor.matmul(
            cps[0:66, hi, c0:512],
            lhsT=v_sb[:, kt, h, :],
            rhs=es[:, hi, c0:512],
            start=(kt == 0),
            stop=(kt == n_kt - 1),
        )


def build_nc(flags=(False, False, False)):
    if flags not in _CACHE:
        _CACHE[flags] = _build(flags)
    return _CACHE[flags]


def make_in_maps(inputs):
    import ml_dtypes
    bf = ml_dtypes.bfloat16
    x = np.asarray(inputs["x"], dtype=np.float32)
    Wq = np.asarray(inputs["Wq"], np.float32)
    Wk = np.asarray(inputs["Wk"], np.float32)
    Wv = np.asarray(inputs["Wv"], np.float32)
    Wo = np.asarray(inputs["Wo"], np.float32)
    bq = np.asarray(inputs["bq"], np.float32)
    bk = np.asarray(inputs["bk"], np.float32)
    bv = np.asarray(inputs["bv"], np.float32)
    bo = np.asarray(inputs["bo"], np.float32)
    gamma = np.asarray(inputs["ln_gamma"], np.float32)
    beta = np.asarray(inputs["ln_beta"], np.float32)

    has_qkv_bias = bool(np.any(bq) or np.any(bk) or np.any(bv))
    has_gamma = not np.allclose(gamma, 1.0)
    has_beta = bool(np.any(beta))
    flags = (has_qkv_bias, has_gamma, has_beta)

    xres_full = x + bo  # residual with output bias folded in
    WoT = np.ascontiguousarray(Wo.T)  # [Dmodel, Dout]

    in_maps = []
    for c in range(NCORES):
        b, r = c // 4, c % 4
        cols = slice(DP * r, DP * (r + 1))
        # rows for this core: for each qt, rows 512*qt + 128*r .. +128
        xres_c = xres_full[b].reshape(4, 4, 128, D)[:, r]
        m = {
            "xT": np.ascontiguousarray(x[b].T.astype(bf)),
            "xres": np.ascontiguousarray(xres_c.astype(bf)),
            "wqT": np.ascontiguousarray(Wq[cols, :].T.astype(bf)),
            "wkT": np.ascontiguousarray(Wk[cols, :].T.astype(bf)),
            "wvT": np.ascontiguousarray(Wv[cols, :].T.astype(bf)),
            "woL": np.ascontiguousarray(WoT[cols, :].astype(bf)),
        }
        if has_qkv_bias:
            m["bqkv"] = np.ascontiguousarray(
                np.stack([bq[cols], bk[cols], bv[cols]])[None].astype(bf))
        if has_gamma:
            m["gamma"] = gamma
        if has_beta:
            m["beta"] = beta
        in_maps.append(m)
    return flags, in_maps


def assemble(results):
    """results: list of per-core dicts with 'out' [4, 128, 1024]."""
    full = np.empty((B, N, D), dtype=np.float32)
    for c in range(NCORES):
        b, r = c // 4, c % 4
        o = results[c]["out"]
        for qt in range(4):
            full[b, NQ * qt + 128 * r:NQ * qt + 128 * (r + 1)] = o[qt]
    return full


def kernel(**inputs):
    from concourse.bass_utils import run_bass_kernel_spmd

    flags, in_maps = make_in_maps(inputs)
    nc = build_nc(flags)
    res = run_bass_kernel_spmd(nc, in_maps, core_ids=list(range(NCORES)))
    return assemble(res.results)



# revision 8
# speedup vs baseline: 1.2289x; 1.2289x over previous
"""Fused causal-attention block (QKV proj + causal softmax attention + out proj
+ residual + LayerNorm) on 8 Trainium2 NeuronCores — bf16 v2.

Sharding: core c -> batch b = c//4, head-group r = c%4 (heads 4r..4r+3, local
model dims 256r..256r+256).  Each core computes Q/K/V for its head group over
its batch's full sequence and block-causal attention (no max subtraction --
scores are O(1)).  Output projection is row-parallel: each core computes the
partial out-proj Y_r = ctx_r @ Wo[:, 256r:256r+256].T for ALL 2048 rows from
its local (normalized) ctx, per 512-row q-tile, overlapped with attention;
a per-q-tile ReduceScatter over the batch's 4 cores sums the partials and
hands each core a 128-row shard, on which it does residual + LayerNorm.
Host reassembles the 8 x [4, 128, 1024] shards.

All matmul operands are bf16 (rel err ~1e-3, gate is 2e-2); PSUM accumulation
is fp32.  The causal mask on diagonal 128x128 blocks is a bf16 upper-tri
multiply applied to exp(scores) on the GPSIMD engine (SBUF-only op, keeps DVE
free).  Softmax denominators come from an all-ones column appended to V; the
[1,512]-per-head reciprocals use the fast custom-DVE approx (~5x faster than
InstReciprocal).  LayerNorm rstd = exp(-0.5*ln(var+eps)) so the Scalar engine
stays on the natural_log_exp activation-table set for the whole kernel (no
table thrash against the attention exp stream).
"""

import numpy as np

B, N, D = 2, 2048, 1024
H, DH = 16, 64
NCORES = 8
HPC = 4          # heads per core
DP = HPC * DH    # 256 local model dims per core
NQ = N // 4      # 512 rows per q-tile
LN_EPS = 1e-5
GROUPS = [[0, 1, 2, 3], [4, 5, 6, 7]]

_CACHE = {}


def _build(flags):
    """Build+compile the Bacc program. flags = (has_qkv_bias, has_gamma, has_beta)."""
    import concourse.bass as bass
    import concourse.bacc as bacc
    import concourse.tile as tile
    from concourse import mybir
    from contextlib import ExitStack

    has_qkv_bias, has_gamma, has_beta = flags
    f32 = mybir.dt.float32
    f32r = mybir.dt.float32r
    bf16 = mybir.dt.bfloat16
    AF = mybir.ActivationFunctionType
    ALU = mybir.AluOpType

    nc = bacc.Bacc(
        trn_type="TRN2",
        target_bir_lowering=False,
        debug=False,
        num_devices=NCORES,
    )

    xT = nc.dram_tensor("xT", [D, N], bf16, kind="ExternalInput").ap()
    xres = nc.dram_tensor("xres", [4, 128, D], bf16, kind="ExternalInput").ap()
    wqT = nc.dram_tensor("wqT", [D, DP], bf16, kind="ExternalInput").ap()
    wkT = nc.dram_tensor("wkT", [D, DP], bf16, kind="ExternalInput").ap()
    wvT = nc.dram_tensor("wvT", [D, DP], bf16, kind="ExternalInput").ap()
    woL = nc.dram_tensor("woL", [DP, D], bf16, kind="ExternalInput").ap()
    out = nc.dram_tensor("out", [4, 128, D], f32, kind="ExternalOutput").ap()
    if has_qkv_bias:
        bqkv = nc.dram_tensor("bqkv", [1, 3, DP], bf16, kind="ExternalInput").ap()
    if has_gamma:
        gamma_d = nc.dram_tensor("gamma", [D], f32, kind="ExternalInput").ap()
    if has_beta:
        beta_d = nc.dram_tensor("beta", [D], f32, kind="ExternalInput").ap()

    # multiplicative causal mask for diagonal blocks: keep k <= q
    # (partition p = k offset, free c = q offset).  Passed as a real input:
    # inline_tensor lowers to a ~10.6us engine-direct copy that also eats an
    # early completion-semaphore slot and gates the first matmul.
    tri_d = nc.dram_tensor("tri", [128, 128], bf16, kind="ExternalInput").ap()

    with tile.TileContext(nc) as tc, ExitStack() as ctx, \
            nc.allow_low_precision(reason="bf16 compute; gate is 2e-2"):
        singles = ctx.enter_context(tc.tile_pool(name="singles", bufs=1))
        qkv_pool = ctx.enter_context(tc.tile_pool(name="qkv", bufs=1))

        # weights, striped k-on-partitions.  wq on sync (fast HWDGE, first in
        # queue) since the very first matmuls need it; wk/wv on gpsimd.
        wq_sb = singles.tile([128, 8, DP], bf16, tag="wq")
        wk_sb = singles.tile([128, 8, DP], bf16, tag="wk")
        wv_sb = singles.tile([128, 8, DP], bf16, tag="wv")
        nc.sync.dma_start(wq_sb, wqT.rearrange("(ko p) m -> p ko m", p=128))
        nc.gpsimd.dma_start(wk_sb, wkT.rearrange("(ko p) m -> p ko m", p=128))
        nc.gpsimd.dma_start(wv_sb, wvT.rearrange("(ko p) m -> p ko m", p=128))

        tri_sb = singles.tile([128, 128], bf16, tag="tri")

        ones_f32 = singles.tile([128, 64], f32, tag="ones_f32")
        nc.vector.memset(ones_f32, 1.0)
        ones64 = singles.tile([1, 64], bf16, tag="ones64")
        nc.vector.tensor_copy(out=ones64, in_=ones_f32[0:1, :])
        eps_sb = singles.tile([128, 1], f32, tag="eps")
        nc.vector.memset(eps_sb, LN_EPS)
        if has_qkv_bias:
            o512f = singles.tile([1, 512], f32, tag="o512f")
            nc.vector.memset(o512f, 1.0)
            ones512 = singles.tile([1, 512], bf16, tag="ones512")
            nc.vector.tensor_copy(out=ones512, in_=o512f)
            bqkv_sb = singles.tile([1, 3, DP], bf16, tag="bqkv")
            nc.scalar.dma_start(bqkv_sb, bqkv)
        if has_gamma:
            gamma_sb = singles.tile([128, D], f32, tag="gamma")
            nc.scalar.dma_start(
                gamma_sb,
                bass.AP(tensor=gamma_d.tensor, offset=gamma_d.offset,
                        ap=[[0, 128]] + gamma_d.ap),
            )
        if has_beta:
            beta_sb = singles.tile([128, D], f32, tag="beta")
            nc.scalar.dma_start(
                beta_sb,
                bass.AP(tensor=beta_d.tensor, offset=beta_d.offset,
                        ap=[[0, 128]] + beta_d.ap),
            )

        # persistent activations
        qT_sb = qkv_pool.tile([128, 2, N], bf16, tag="qT")   # Q^T [d'(256), n]
        kT_sb = qkv_pool.tile([128, 2, N], bf16, tag="kT")   # K^T [d'(256), n]
        v_sb = qkv_pool.tile([128, 16, HPC, DH + 2], bf16, tag="v")  # V + ones
        ctx_sb = qkv_pool.tile([128, 2, N], bf16, tag="ctxT")  # normalized ctx^T
        # The AV stationary reads the FULL 66-wide inner dim (odd-width
        # partial slices break HW ldweights addressing): zero the tile, set
        # the denominator ones column; col 65 stays 0 so its output row is
        # junk-free and ignored.
        nc.vector.memset(v_sb, 0.0)
        nc.vector.tensor_copy(
            out=v_sb[:, :, :, DH:DH + 1],
            in_=ones_f32.rearrange("p (a b c) -> p a b c", a=16, b=4))

        # ---------------- Phase 1: QKV projections ----------------
        with tc.tile_pool(name="xt", bufs=1) as xt_pool, \
             tc.tile_pool(name="p1qk", bufs=4, space="PSUM") as p1qk, \
             tc.tile_pool(name="p1v", bufs=3, space="PSUM") as p1v:
            xT_sb = xt_pool.tile([128, 8, N], bf16, tag="xT")
            xT_r = xT.rearrange("(ko p) n -> p ko n", p=128)
            # 4 column chunks in nt-consumption order, spread over 4 queues so
            # issue costs overlap; chunk0 on scalar (fast HWDGE, first in its
            # queue) so the first Q matmuls are gated only on wq+chunk0
            dma_engs = [nc.scalar, nc.sync, nc.scalar, nc.sync]
            for ci in range(4):
                dma_engs[ci].dma_start(
                    xT_sb[:, :, 512 * ci:512 * (ci + 1)],
                    xT_r[:, :, 512 * ci:512 * (ci + 1)])
            nc.gpsimd.dma_start(tri_sb, tri_d)

            for wsb, dst, bidx in ((wq_sb, qT_sb, 0), (wk_sb, kT_sb, 1)):
                for dt_ in range(2):
                    for nt in range(4):
                        ps = p1qk.tile([128, 512], f32, tag="qk")
                        for ko in range(8):
                            nc.tensor.matmul(
                                ps,
                                lhsT=wsb[:, ko, 128 * dt_:128 * dt_ + 128],
                                rhs=xT_sb[:, ko, 512 * nt:512 * nt + 512],
                                start=(ko == 0),
                                stop=(ko == 7 and not has_qkv_bias),
                            )
                        if has_qkv_bias:
                            nc.tensor.matmul(
                                ps,
                                lhsT=bqkv_sb[:, bidx, 128 * dt_:128 * dt_ + 128],
                                rhs=ones512,
                                start=False, stop=True,
                            )
                        nc.vector.tensor_copy(
                            out=dst[:, dt_, 512 * nt:512 * (nt + 1)], in_=ps)

            for nt in range(16):
                ps = p1v.tile([128, DP], f32, tag="v")
                for ko in range(8):
                    nc.tensor.matmul(
                        ps,
                        lhsT=xT_sb[:, ko, 128 * nt:128 * nt + 128],
                        rhs=wv_sb[:, ko],
                        start=(ko == 0),
                        stop=(ko == 7 and not has_qkv_bias),
                    )
                if has_qkv_bias:
                    nc.tensor.matmul(
                        ps,
                        lhsT=ones512[:, 0:128],
                        rhs=bqkv_sb[:, 2, :],
                        start=False, stop=True,
                    )
                nc.vector.tensor_copy(
                    out=v_sb[:, nt, :, 0:DH],
                    in_=ps.rearrange("p (h d) -> p h d", h=HPC))

        # local Wo^T slice + residual rows + LN params: small loads that
        # overlap attention
        wo_pool = ctx.enter_context(tc.tile_pool(name="wop", bufs=1))
        wo_sb = wo_pool.tile([128, 2, D], bf16, tag="wo")
        nc.scalar.dma_start(wo_sb, woL.rearrange("(c p) d -> p c d", p=128))
        xres_sb = wo_pool.tile([128, 4, D], bf16, tag="xres")
        nc.gpsimd.dma_start(
            xres_sb, xres.rearrange("a p d -> p a d"))

        dram_pool = ctx.enter_context(tc.tile_pool(name="dram", bufs=1,
                                                   space="DRAM"))
        y_dram = [dram_pool.tile([NQ, D], bf16, tag=f"y{qt}", name=f"y{qt}")
                  for qt in range(4)]
        yr_dram = [dram_pool.tile([128, D], bf16, tag=f"yr{qt}",
                                  name=f"yr{qt}")
                   for qt in range(4)]

        # ------------- Phase 2: attention + overlapped out-proj -------------
        with tc.tile_pool(name="es", bufs=6) as es_pool, \
             tc.tile_pool(name="nrm", bufs=4) as nrm_pool, \
             tc.tile_pool(name="ysb", bufs=2) as y_pool, \
             tc.tile_pool(name="ln", bufs=2) as ln_pool, \
             tc.tile_pool(name="sps", bufs=2, space="PSUM") as sps_pool, \
             tc.tile_pool(name="cps", bufs=2, space="PSUM") as cps_pool:

            from concourse.dve_ops import (
                RECIP_APPROX_FAST_CONSTS,
                RECIPROCAL_APPROX_FAST,
            )

            def emit_recip(qt, hp, cps):
                # Evacuate the denominator row to partition 0 of SBUF (the
                # custom-DVE op ignores input base partitions on HW), then
                # 1/denoms for both heads in one fast custom-DVE op (~5x
                # faster than InstReciprocal), cast to bf16 for the
                # broadcast matmul.
                den = nrm_pool.tile([1, 2, 512], f32, tag="den",
                                    name=f"den_{qt}_{hp}")
                nc.vector.tensor_copy(out=den, in_=cps[64:65, :, :])
                recf = nrm_pool.tile([1, 2, 512], f32, tag="recf",
                                     name=f"recf_{qt}_{hp}")
                cc = RECIP_APPROX_FAST_CONSTS
                nc.vector._custom_dve(
                    RECIPROCAL_APPROX_FAST, out=recf, in0=den,
                    s0=cc["s0"], s1=cc["s1"], imm2=cc["imm2"])
                rec = nrm_pool.tile([1, 2, 512], bf16, tag="rec",
                                    name=f"rec_{qt}_{hp}")
                nc.vector.tensor_copy(out=rec, in_=recf)
                return rec

            def emit_normalize(qt, hp, cps, rec):
                bc_full = sps_pool.tile([128, 2, 512], f32, tag="s",
                                        name=f"bc_{qt}_{hp}")
                bcs = nrm_pool.tile([64, 2, 512], bf16, tag="bcs",
                                    name=f"bcs_{qt}_{hp}")
                for hi in range(2):
                    nc.tensor.matmul(
                        bc_full[0:64, hi, :], lhsT=ones64,
                        rhs=rec[0:1, hi, :],
                        start=True, stop=True)
                    nc.vector.tensor_copy(out=bcs[:, hi, :],
                                          in_=bc_full[0:64, hi, :])
                for hi, h in enumerate((2 * hp, 2 * hp + 1)):
                    ph = 64 * (h % 2)
                    nc.vector.tensor_mul(
                        out=ctx_sb[ph:ph + 64, hp, 512 * qt:512 * (qt + 1)],
                        in0=cps[0:64, hi, :], in1=bcs[:, hi, :])

            def emit_outproj(qt):
                # partial out-proj for this q-tile from local normalized ctx;
                # stage to DRAM and ReduceScatter across the batch group
                y_sb = y_pool.tile([128, 4, D], bf16, tag="y",
                                   name=f"y_sb{qt}")
                for qc in range(4):
                    ops = sps_pool.tile([128, 2, 512], f32, tag="s",
                                        name=f"op_{qt}_{qc}")
                    for Dt in range(2):
                        for hp in range(2):
                            nc.tensor.matmul(
                                ops[:, Dt, :],
                                lhsT=ctx_sb[:, hp,
                                            512 * qt + 128 * qc:
                                            512 * qt + 128 * qc + 128],
                                rhs=wo_sb[:, hp, 512 * Dt:512 * Dt + 512],
                                start=(hp == 0), stop=(hp == 1),
                            )
                    nc.vector.tensor_copy(
                        out=y_sb[:, qc, :],
                        in_=ops.rearrange("p a b -> p (a b)"))
                nc.sync.dma_start(
                    y_dram[qt].rearrange("(qc p) d -> p qc d", p=128), y_sb)
                nc.gpsimd.collective_compute(
                    "ReduceScatter", ALU.add,
                    replica_groups=GROUPS,
                    ins=[y_dram[qt][:, :]],
                    outs=[yr_dram[qt][:, :]],
                )

            def emit_lntail(qt):
                yr_sb = ln_pool.tile([128, D], bf16, tag="yr",
                                     name=f"yr_sb{qt}")
                nc.sync.dma_start(yr_sb, yr_dram[qt])
                yt = ln_pool.tile([128, D], f32, tag="yt", name=f"yt{qt}")
                nc.vector.tensor_add(out=yt, in0=yr_sb, in1=xres_sb[:, qt])
                st = ln_pool.tile([128, 2, 6], f32, tag="st", name=f"st{qt}")
                nc.vector.bn_stats(out=st[:, 0], in_=yt[:, 0:512])
                nc.vector.bn_stats(out=st[:, 1], in_=yt[:, 512:1024])
                mv = ln_pool.tile([128, 2], f32, tag="mv", name=f"mv{qt}")
                nc.vector.bn_aggr(out=mv, in_=st)
                # rstd = exp(-0.5*ln(var+eps)): stays on the natural_log_exp
                # activation table set (same set as the attention exp)
                lnt = ln_pool.tile([128, 1], f32, tag="lnt", name=f"lnt{qt}")
                rstd = ln_pool.tile([128, 1], f32, tag="rstd",
                                    name=f"rstd{qt}")
                nc.scalar.activation(out=lnt, in_=mv[:, 1:2], func=AF.Ln,
                                     bias=eps_sb, scale=1.0)
                nc.scalar.activation(out=rstd, in_=lnt, func=AF.Exp,
                                     scale=-0.5)
                nc.vector.tensor_scalar(
                    out=yt, in0=yt, scalar1=mv[:, 0:1], scalar2=rstd,
                    op0=ALU.subtract, op1=ALU.mult)
                if has_gamma:
                    nc.vector.tensor_mul(out=yt, in0=yt, in1=gamma_sb)
                if has_beta:
                    nc.vector.tensor_add(out=yt, in0=yt, in1=beta_sb)
                nc.sync.dma_start(out[qt], yt)

            # deferred-work queue: (min_pair_idx, fn) emitted at staggered
            # points inside later pairs' kt loops so the in-order PE never
            # drains behind DVE/collective latencies
            deferred = []

            def pump(pidx):
                for i, (minp, fn) in enumerate(deferred):
                    if minp <= pidx:
                        deferred.pop(i)
                        fn()
                        return

            for qt in range(4):
                for hp in range(2):
                    pidx = 2 * qt + hp
                    heads = (2 * hp, 2 * hp + 1)
                    cps = cps_pool.tile([128, 2, 512], f32, tag="ctx",
                                        name=f"cps_{qt}_{hp}")
                    n_kt = 4 * qt + 4
                    milestones = sorted({2, n_kt // 2, n_kt - 2})
                    pend = []
                    for kt in range(n_kt):
                        j = kt - 4 * qt
                        c0 = 128 * j if j > 0 else 0
                        sp = sps_pool.tile([128, 2, 512], f32, tag="s",
                                           name=f"sp_{qt}_{hp}_{kt}")
                        for hi, h in enumerate(heads):
                            ph = 64 * (h % 2)
                            nc.tensor.matmul(
                                sp[:, hi, c0:512],
                                lhsT=kT_sb[ph:ph + 64, hp,
                                           128 * kt:128 * kt + 128],
                                rhs=qT_sb[ph:ph + 64, hp,
                                          512 * qt + c0:512 * (qt + 1)],
                                start=True,
                                stop=True,
                            )
                        es = es_pool.tile([128, 2, 512], bf16, tag="es")
                        nc.scalar.activation(
                            out=es[:, :, c0:512],
                            in_=sp[:, :, c0:512],
                            func=AF.Exp, scale=0.125,
                        )
                        if j >= 0:
                            # masks on DVE: the gpsimd queue carries the RS
                            # triggers, whose waits must not stall masking
                            for hi in range(2):
                                nc.vector.tensor_mul(
                                    out=es[:, hi, 128 * j:128 * j + 128],
                                    in0=es[:, hi, 128 * j:128 * j + 128],
                                    in1=tri_sb)
                        pend.append((es, kt, c0))
                        if len(pend) > 2:
                            _emit_av(nc, cps, v_sb, heads, pend.pop(0), n_kt)
                        if kt in milestones:
                            pump(pidx)
                    while pend:
                        _emit_av(nc, cps, v_sb, heads, pend.pop(0), n_kt)
                    rec = emit_recip(qt, hp, cps)
                    deferred.append(
                        (pidx + 1,
                         (lambda a, b, c, d: lambda: emit_normalize(a, b, c, d))(
                             qt, hp, cps, rec)))
                    if hp == 1:
                        deferred.append(
                            (pidx + 1, (lambda a: lambda: emit_outproj(a))(qt)))
            while deferred:
                deferred.sort(key=lambda it: it[0])
                _, fn = deferred.pop(0)
                fn()
            # all residual+LN tails at the end: RS(0..2) wires are long done,
            # so no mid-stream engine ever blocks on a collective semaphore
            for qt in range(4):
                emit_lntail(qt)

    # Pin every activation to the natural_log_exp table set (covers exp, ln,
    # copy) so the Scalar engine never swaps tables between the attention exp
    # stream and the LayerNorm rstd (each swap costs ~1.5us + a pipe drain).
    import concourse.bacc as bacc_mod
    orig_tables = bacc_mod.get_activation_tables
    # keep dict insertion order (act_func_set_id is positional) but leave
    # functions only in the one set we want chosen
    bacc_mod.get_activation_tables = lambda arch: {
        k: (v if k == "natural_log_exp_and_others" else set())
        for k, v in orig_tables(arch).items()
    }
    try:
        nc.compile()
    finally:
        bacc_mod.get_activation_tables = orig_tables
    return nc


def _emit_av(nc, cps, v_sb, heads, pend_item, n_kt):
    es, kt, c0 = pend_item
    for hi, h in enumerate(heads):
        nc.tensor.matmul(
            cps[0:66, hi, c0:512],
            lhsT=v_sb[:, kt, h, :],
            rhs=es[:, hi, c0:512],
            start=(kt == 0),
            stop=(kt == n_kt - 1),
        )


def build_nc(flags=(False, False, False)):
    if flags not in _CACHE:
        _CACHE[flags] = _build(flags)
    return _CACHE[flags]


def make_in_maps(inputs):
    import ml_dtypes
    bf = ml_dtypes.bfloat16
    x = np.asarray(inputs["x"], dtype=np.float32)
    Wq = np.asarray(inputs["Wq"], np.float32)
    Wk = np.asarray(inputs["Wk"], np.float32)
    Wv = np.asarray(inputs["Wv"], np.float32)
    Wo = np.asarray(inputs["Wo"], np.float32)
    bq = np.asarray(inputs["bq"], np.float32)
    bk = np.asarray(inputs["bk"], np.float32)
    bv = np.asarray(inputs["bv"], np.float32)
    bo = np.asarray(inputs["bo"], np.float32)
    gamma = np.asarray(inputs["ln_gamma"], np.float32)
    beta = np.asarray(inputs["ln_beta"], np.float32)

    has_qkv_bias = bool(np.any(bq) or np.any(bk) or np.any(bv))
    has_gamma = not np.allclose(gamma, 1.0)
    has_beta = bool(np.any(beta))
    flags = (has_qkv_bias, has_gamma, has_beta)

    xres_full = x + bo  # residual with output bias folded in
    WoT = np.ascontiguousarray(Wo.T)  # [Dmodel, Dout]
    tri_np = np.ascontiguousarray(
        np.triu(np.ones((128, 128), np.float32)).astype(bf))

    in_maps = []
    for c in range(NCORES):
        b, r = c // 4, c % 4
        cols = slice(DP * r, DP * (r + 1))
        # rows for this core: for each qt, rows 512*qt + 128*r .. +128
        xres_c = xres_full[b].reshape(4, 4, 128, D)[:, r]
        m = {
            "xT": np.ascontiguousarray(x[b].T.astype(bf)),
            "xres": np.ascontiguousarray(xres_c.astype(bf)),
            "wqT": np.ascontiguousarray(Wq[cols, :].T.astype(bf)),
            "wkT": np.ascontiguousarray(Wk[cols, :].T.astype(bf)),
            "wvT": np.ascontiguousarray(Wv[cols, :].T.astype(bf)),
            "woL": np.ascontiguousarray(WoT[cols, :].astype(bf)),
            "tri": tri_np,
        }
        if has_qkv_bias:
            m["bqkv"] = np.ascontiguousarray(
                np.stack([bq[cols], bk[cols], bv[cols]])[None].astype(bf))
        if has_gamma:
            m["gamma"] = gamma
        if has_beta:
            m["beta"] = beta
        in_maps.append(m)
    return flags, in_maps


def assemble(results):
    """results: list of per-core dicts with 'out' [4, 128, 1024]."""
    full = np.empty((B, N, D), dtype=np.float32)
    for c in range(NCORES):
        b, r = c // 4, c % 4
        o = results[c]["out"]
        for qt in range(4):
            full[b, NQ * qt + 128 * r:NQ * qt + 128 * (r + 1)] = o[qt]
    return full


def kernel(**inputs):
    from concourse.bass_utils import run_bass_kernel_spmd

    flags, in_maps = make_in_maps(inputs)
    nc = build_nc(flags)
    res = run_bass_kernel_spmd(nc, in_maps, core_ids=list(range(NCORES)))
    return assemble(res.results)

